# revision 8
# baseline (speedup 1.0000x reference)
"""CondConv (routing -> per-sample mixed 3x3 conv -> frozen BN -> ReLU -> residual)
on 8 Trainium2 NeuronCores, data-parallel over batch (4 samples/core).

Algorithm: 1-D Winograd F(2,3) along H. The 3x3 conv becomes, per output
row-pair p, four "plane" convolutions (u = 0..3) that are 1x3 convs along W
against H-transformed inputs:
  S0 = x[2p]   - x[2p+2]
  S1 = x[2p+1] + x[2p+2]
  S2 = x[2p+2] - x[2p+1]
  S3 = x[2p+1] - x[2p+3]          (rows of the padded image)
  M_u[o,p,w] = sum_kj sum_c U_u[kj;c,o] * S_u[c,p,w+kj-1]
  y[2p]   = M0 + M1 + M2
  y[2p+1] = M1 - M2 - M3
This cuts tensor-engine columns 1.5x vs direct 9-tap conv (24 vs 36
matmuls per output chunk at the same free-dim).

Host precomputes the G-transform of the expert bank along ki (all +-1/2
coeffs) and folds the BN scale gamma' = gamma/sqrt(var+eps) into it (both
commute with the routing mix, which stays linear). Device output is bf16
(cast to fp32 on host); BN bias beta' is applied by the ACT Relu.

Engine split per core: PE runs 768 FD-392 matmuls; DVE does routing dot,
weight mixing (stt chains), M-plane combines and the residual adds; ACT does
GAP (Copy+accum), sigmoid, M1/M2 PSUM evacuation and bias+ReLU; GPSIMD does
the routing all-reduce and the S-plane transforms.
"""

import threading

import ml_dtypes
import numpy as np

import concourse.bass as bass
import concourse.mybir as mybir
import concourse.tile as tile
from concourse import bacc, bass_isa
from concourse.bass_utils import run_bass_kernel_spmd

F32 = mybir.dt.float32
BF16 = mybir.dt.bfloat16
AX = mybir.AxisListType
OP = mybir.AluOpType
AF = mybir.ActivationFunctionType

N_CORES = 8
B, CIN, COUT, H, W, KS, E = 32, 256, 256, 56, 56, 3, 4
BPC = B // N_CORES  # samples per core
CT = CIN // 128     # cin partition tiles
OTN = COUT // 128   # cout partition tiles
P = H // 2          # output row pairs
PC = 7              # row pairs per chunkset
NCS = P // PC       # chunksets per (sample, cout tile)
HH = (H + 2) // 2   # padded rows stored as [HH, 2] (row = 2*hh + par)
WP = W + 2          # width zero-padded
K12 = 12            # u*3 + kj
NF = PC * W         # matmul free dim = 392
BN_EPS = 1e-5


def build_bass():
    nc = bacc.Bacc("TRN2", target_bir_lowering=False, debug=False)

    x_d = nc.dram_tensor("x", [BPC, CIN, HH, 2, WP], BF16, kind="ExternalInput")
    wt_d = nc.dram_tensor("wt", [E, 128, OTN, CT, K12, 128], BF16,
                          kind="ExternalInput")
    rwt_d = nc.dram_tensor("rwt", [128, CT, E], F32, kind="ExternalInput")
    rb_d = nc.dram_tensor("rb", [E], F32, kind="ExternalInput")
    bnb_d = nc.dram_tensor("bnb", [128, OTN], F32, kind="ExternalInput")
    y_d = nc.dram_tensor("y", [BPC, COUT, P, 2, W], BF16, kind="ExternalOutput")

    x_ap = x_d.ap()
    wt_ap = wt_d.ap()
    rwt_ap = rwt_d.ap()
    rb_ap = rb_d.ap()
    bnb_ap = bnb_d.ap()
    y_ap = y_d.ap()

    with tile.TileContext(nc) as tc:
        with (
            tc.tile_pool(name="wbp", bufs=1) as wbp,
            tc.tile_pool(name="xpp", bufs=1) as xpp,
            tc.tile_pool(name="mwp", bufs=1) as mwp,
            tc.tile_pool(name="ssp", bufs=1) as ssp,
            tc.tile_pool(name="otp", bufs=3) as otp,
            tc.tile_pool(name="mxs", bufs=1) as mxs,
            tc.tile_pool(name="snp", bufs=1) as snp,
            tc.tile_pool(name="smp", bufs=4) as smp,
            tc.tile_pool(name="psp", bufs=2, space="PSUM") as psp,
        ):
            # ---- persistent tiles ----
            wb = [wbp.tile([128, OTN, CT, K12, 128], BF16, name=f"wb{e}",
                           tag=f"wb{e}") for e in range(E)]
            xp = [xpp.tile([128, CT, HH, 2, WP], BF16, name=f"xp{i}",
                           tag=f"xp{i}") for i in range(2)]
            mw = [mwp.tile([128, OTN, CT, K12, 128], BF16, name=f"mw{i}",
                           tag=f"mw{i}") for i in range(2)]
            sp = [ssp.tile([128, CT, 4, P, WP], BF16, name=f"sp{i}",
                           tag=f"sp{i}") for i in range(2)]
            rwt_sb = snp.tile([128, CT, E], F32, name="rwt", tag="rwt")
            rb_bc = snp.tile([128, E], F32, name="rb_bc", tag="rb_bc")
            bnb_sb = snp.tile([128, OTN], F32, name="bnb", tag="bnb")
            zeros1 = snp.tile([128, 1], F32, name="zeros1", tag="zeros1")
            pscr = snp.tile([128, HH * 2 * WP], BF16, name="pscr", tag="pscr")
            warm_w = snp.tile([128, 128], BF16, name="warm_w", tag="warm_w")
            warm_x = snp.tile([128, NF], BF16, name="warm_x", tag="warm_x")
            mxc = [mxs.tile([128, CT * K12 * 128], BF16, name=f"mxc{e}",
                            tag=f"mxc{e}") for e in range(1, E)]

            # ---- preamble DMAs in priority order ----
            # sample 0 x split 4 ways across engine queues
            nc.sync.dma_start(out=xp[0][:, 0, 0:15], in_=x_ap[0, 0:128, 0:15])
            nc.scalar.dma_start(out=xp[0][:, 0, 15:29], in_=x_ap[0, 0:128, 15:29])
            nc.sync.dma_start(out=xp[0][:, 1, 0:15], in_=x_ap[0, 128:256, 0:15])
            nc.gpsimd.dma_start(out=xp[0][:, 1, 15:29],
                                in_=x_ap[0, 128:256, 15:29])
            nc.sync.dma_start(out=rwt_sb, in_=rwt_ap[:, :, :])  # [128, CT, E]
            nc.sync.dma_start(
                out=rb_bc,
                in_=bass.AP(tensor=rb_ap.tensor, offset=0, ap=[[0, 128], [1, E]]))
            nc.sync.dma_start(out=bnb_sb, in_=bnb_ap[:, :])
            # expert bank, oi=0 halves first (mixing consumes them first)
            for oi in range(OTN):
                for e in range(E):
                    nc.sync.dma_start(out=wb[e][:, oi], in_=wt_ap[e, :, oi])

            nc.vector.memset(zeros1, 0.0)
            nc.vector.memset(warm_w, 0.0)
            nc.vector.memset(warm_x, 0.0)

            # warm-up matmuls: keep the PE HAM window busy until the first
            # real matmul (~18us in); batch 2 is dependency-gated on routing.
            warm_ps = psp.tile([128, 2, 512], F32, name="warm_ps", tag="qa")
            for _ in range(36):
                nc.tensor.matmul(warm_ps[:, 0, 0:NF], lhsT=warm_w, rhs=warm_x,
                                 start=True, stop=True)

            def routing(s):
                """GAP -> dot -> all-reduce -> sigmoid; returns rr [128,E]."""
                i = s % 2
                pooled = smp.tile([128, CT], F32, name=f"pool{s}", tag="pool")
                nc.scalar.activation(out=pscr, in_=xp[i][:, 0], func=AF.Copy,
                                     accum_out=pooled[:, 0:1])
                nc.scalar.activation(out=pscr, in_=xp[i][:, 1], func=AF.Copy,
                                     accum_out=pooled[:, 1:2])
                prod = smp.tile([128, E], F32, name=f"prod{s}", tag="prod")
                nc.vector.tensor_scalar_mul(prod, rwt_sb[:, 0], pooled[:, 0:1])
                nc.vector.scalar_tensor_tensor(out=prod, in0=rwt_sb[:, 1],
                                               scalar=pooled[:, 1:2], in1=prod,
                                               op0=OP.mult, op1=OP.add)
                lg = smp.tile([128, E], F32, name=f"lg{s}", tag="lg")
                nc.gpsimd.partition_all_reduce(lg, prod, channels=128,
                                               reduce_op=bass_isa.ReduceOp.add)
                nc.vector.scalar_tensor_tensor(out=lg, in0=lg,
                                               scalar=1.0 / (H * W), in1=rb_bc,
                                               op0=OP.mult, op1=OP.add)
                rr = smp.tile([128, E], F32, name=f"rr{s}", tag="rr")
                nc.scalar.activation(out=rr, in_=lg, func=AF.Sigmoid,
                                     bias=zeros1)
                return rr

            def mix(s, rr, oi, t=None):
                """mw[oi(,t)] = sum_e rr[e] * wb[e][oi(,t)]: tensor_scalar
                scalings (4x mode) + dense bf16 adds (2x) -- scalar_tensor_
                tensor only runs at 1x, so the scale+add chain is slower."""
                i = s % 2
                if t is None:
                    dst = mw[i][:, oi]
                    srcs = [wb[e][:, oi] for e in range(E)]
                    n = CT * K12 * 128
                else:
                    dst = mw[i][:, oi, t]
                    srcs = [wb[e][:, oi, t] for e in range(E)]
                    n = K12 * 128
                nc.vector.tensor_scalar_mul(dst, srcs[0], rr[:, 0:1])
                for e in range(1, E):
                    nc.vector.tensor_scalar_mul(mxc[e - 1][:, 0:n], srcs[e],
                                                rr[:, e:e + 1])
                for e in range(1, E):
                    nc.vector.tensor_add(dst, dst, mxc[e - 1][:, 0:n])

            def s_planes(s, p0, p1, eng):
                """S planes for row pairs [p0,p1) on DVE ('v') or GPSIMD ('g').
                DVE gets 2D per-cin-tile slices (3-level strided APs run ~4x
                slower there); GPSIMD's software AGU is rank-agnostic."""
                i = s % 2
                v = nc.vector if eng == "v" else nc.gpsimd
                tsl = range(CT) if eng == "v" else [slice(None)]
                for t in tsl:
                    xeA = xp[i][:, t, p0:p1, 0, :]
                    xeB = xp[i][:, t, p0 + 1:p1 + 1, 0, :]
                    xoA = xp[i][:, t, p0:p1, 1, :]
                    xoB = xp[i][:, t, p0 + 1:p1 + 1, 1, :]
                    v.tensor_sub(sp[i][:, t, 0, p0:p1, :], xeA, xeB)
                    v.tensor_add(sp[i][:, t, 1, p0:p1, :], xoA, xeB)
                    v.tensor_sub(sp[i][:, t, 2, p0:p1, :], xeB, xoA)
                    v.tensor_sub(sp[i][:, t, 3, p0:p1, :], xoA, xoB)

            def conv(s, oi):
                """One cout tile of sample s: 4 chunksets of 24 matmuls + the
                Winograd output combine + bias/ReLU + residual + store.
                M-planes land pairwise in 2-bank PSUM tiles (bank-padded to
                512 fp32) so ACT evacuates each pair in one FD-784 Copy and
                every DVE combine is a dense bf16 2x-mode op."""
                i = s % 2
                for cs in range(NCS):
                    p0 = cs * PC
                    qa = psp.tile([128, 2, 512], F32, name=f"qa{s}{oi}{cs}",
                                  tag="qa")     # (M0, M3)
                    qb = psp.tile([128, 2, 512], F32, name=f"qb{s}{oi}{cs}",
                                  tag="qb")     # (M1, M2)
                    dsts = [qa[:, 0, 0:NF], qb[:, 0, 0:NF],
                            qb[:, 1, 0:NF], qa[:, 1, 0:NF]]
                    for t in range(CT):
                        for u in range(4):
                            for kj in range(3):
                                nc.tensor.matmul(
                                    dsts[u],
                                    lhsT=mw[i][:, oi, t, u * 3 + kj, :],
                                    rhs=sp[i][:, t, u, p0:p0 + PC, kj:kj + W],
                                    start=(t == 0 and kj == 0),
                                    stop=(t == CT - 1 and kj == 2))
                    eva = otp.tile([128, 2, PC, W], BF16, name=f"ea{s}{oi}{cs}",
                                   tag="eva")
                    nc.scalar.activation(out=eva, in_=qa[:, :, 0:NF],
                                         func=AF.Copy)
                    evb = otp.tile([128, 2, PC, W], BF16, name=f"eb{s}{oi}{cs}",
                                   tag="evb")
                    nc.scalar.activation(out=evb, in_=qb[:, :, 0:NF],
                                         func=AF.Copy)
                    te = otp.tile([128, PC, W], BF16, name=f"te{s}{oi}{cs}",
                                  tag="te")
                    nc.vector.tensor_add(te, eva[:, 0], evb[:, 0])
                    to = otp.tile([128, PC, W], BF16, name=f"to{s}{oi}{cs}",
                                  tag="to")
                    nc.vector.tensor_sub(to, evb[:, 0], evb[:, 1])
                    y2 = otp.tile([128, 2, PC, W], BF16, name=f"y2{s}{oi}{cs}",
                                  tag="y2")
                    nc.vector.tensor_add(y2[:, 0], te, evb[:, 1])
                    nc.vector.tensor_sub(y2[:, 1], to, eva[:, 1])
                    rl = otp.tile([128, 2, PC, W], BF16, name=f"rl{s}{oi}{cs}",
                                  tag="rl")
                    nc.scalar.activation(out=rl, in_=y2, func=AF.Relu,
                                         bias=bnb_sb[:, oi:oi + 1])
                    ob = otp.tile([128, PC, 2, W], BF16, name=f"ob{s}{oi}{cs}",
                                  tag="ob")
                    nc.vector.tensor_add(ob[:, :, 0, :], rl[:, 0],
                                         xp[i][:, oi, p0:p0 + PC, 1, 1:1 + W])
                    rode = nc.gpsimd if oi == 0 else nc.vector
                    rode.tensor_add(ob[:, :, 1, :], rl[:, 1],
                                    xp[i][:, oi, p0 + 1:p0 + 1 + PC, 0,
                                         1:1 + W])
                    nc.sync.dma_start(
                        out=y_ap[s, oi * 128:(oi + 1) * 128, p0:p0 + PC, :, :],
                        in_=ob)

            # ---- sample 0 prep: latency-optimized ----
            s_planes(0, 0, 7, "g")          # rows for chunkset 0 on GPSIMD
            rr0 = routing(0)
            # gated warm-ups bridge the routing tail
            nc.vector.tensor_copy(warm_x[0:1, 0:E], rr0[0:1, :])
            for _ in range(12):
                nc.tensor.matmul(warm_ps[:, 0, 0:NF], lhsT=warm_w, rhs=warm_x,
                                 start=True, stop=True)
            mix(0, rr0, 0, t=0)
            mix(0, rr0, 0, t=1)
            s_planes(0, 7, 14, "v")
            s_planes(0, 14, 28, "v")
            rr_cur = rr0

            for s in range(BPC):
                if s + 1 < BPC:
                    j = (s + 1) % 2
                    nc.sync.dma_start(out=xp[j][:, 0], in_=x_ap[s + 1, 0:128])
                    nc.scalar.dma_start(out=xp[j][:, 1],
                                        in_=x_ap[s + 1, 128:256])
                conv(s, 0)
                mix(s, rr_cur, 1)           # oi=1 for current sample
                if s + 1 < BPC:
                    rr_nxt = routing(s + 1)
                    mix(s + 1, rr_nxt, 0)
                    s_planes(s + 1, 0, 14, "g")
                    s_planes(s + 1, 14, 28, "g")
                    rr_cur = rr_nxt
                conv(s, 1)

    nc.compile()
    return nc


_CACHE = {}
_LOCK = threading.Lock()


def prepare_in_maps(inputs):
    """Host-side layout prep: BN fold + Winograd G-transform + sharding."""
    x = np.asarray(inputs["x"], dtype=np.float32)
    route_w = np.asarray(inputs["route_w"], dtype=np.float32)
    route_b = np.ascontiguousarray(np.asarray(inputs["route_b"], np.float32))
    expert_w = np.asarray(inputs["expert_w"], dtype=np.float32)
    bn_gamma = np.asarray(inputs["bn_gamma"], dtype=np.float32)
    bn_beta = np.asarray(inputs["bn_beta"], dtype=np.float32)
    bn_mean = np.asarray(inputs["bn_mean"], dtype=np.float32)
    bn_var = np.asarray(inputs["bn_var"], dtype=np.float32)

    inv = bn_gamma / np.sqrt(bn_var + BN_EPS)
    beta_p = bn_beta - bn_mean * inv
    bank = expert_w * inv[None, :, None, None, None]      # [E,O,I,ki,kj]
    w0, w1, w2 = bank[:, :, :, 0], bank[:, :, :, 1], bank[:, :, :, 2]
    U = np.stack([w0, (w0 + w1 + w2) * 0.5, (w0 - w1 + w2) * 0.5, w2],
                 axis=3)                                   # [E,O,I,u,kj]
    # device layout wt[e, p, oi, t, u*3+kj, o'] = U[e, oi*128+o', p+128t, u, kj]
    Ur = U.reshape(E, OTN, 128, CT, 128, 4, 3)
    wt = np.ascontiguousarray(Ur.transpose(0, 4, 1, 3, 5, 6, 2)
                              .reshape(E, 128, OTN, CT, K12, 128))
    wt = wt.astype(ml_dtypes.bfloat16)

    rwt = np.ascontiguousarray(route_w.T.reshape(CT, 128, E)
                               .transpose(1, 0, 2))
    bnb = np.ascontiguousarray(beta_p.reshape(OTN, 128).T)

    xpad = np.zeros((B, CIN, H + 2, WP), dtype=ml_dtypes.bfloat16)
    xpad[:, :, 1:H + 1, 1:W + 1] = x.astype(ml_dtypes.bfloat16)
    xpad = xpad.reshape(B, CIN, HH, 2, WP)

    return [
        {"x": np.ascontiguousarray(xpad[c * BPC:(c + 1) * BPC]),
         "wt": wt, "rwt": rwt, "rb": route_b, "bnb": bnb}
        for c in range(N_CORES)
    ]


def _get_nc():
    with _LOCK:
        if "nc" not in _CACHE:
            _CACHE["nc"] = build_bass()
        return _CACHE["nc"]


def kernel(**inputs):
    in_maps = prepare_in_maps(inputs)
    nc = _get_nc()
    res = run_bass_kernel_spmd(nc, in_maps, core_ids=list(range(N_CORES)))
    y = np.concatenate([np.asarray(r["y"]) for r in res.results], axis=0)
    return y.reshape(B, COUT, H, W).astype(np.float32)


# revision 10
# speedup vs baseline: 1.3181x; 1.3181x over previous
"""CondConv (routing -> per-sample mixed 3x3 conv -> frozen BN -> ReLU -> residual)
on 8 Trainium2 NeuronCores, data-parallel over batch (4 samples/core).

Per core:
  - expert bank resident in SBUF as bf16, host-pretransposed to
    [cin, cout-half, kk, 128] so each cout half is contiguous
  - routing: GAP (DVE reduce) -> dot with route_w (DVE + gpsimd partition
    all-reduce; keeps the PE queue free for conv matmuls) -> sigmoid (ACT)
  - per-sample mixed kernel: DVE scalar_tensor_tensor accumulation in bf16,
    split per cout half so the first conv starts after half the mixing
  - conv: per output tile, 18 accumulating bf16 matmuls (2 cin tiles x 3x3
    taps; fp32 PSUM) against width-padded bf16 images; moving dim = 8 rows
    x 56 cols = 448; bf16 weight loads get FWL so LDW hides under the stream
  - BN(frozen)+ReLU fused into the ACT PSUM evacuation, residual add on DVE,
    fp32 output
"""

import threading

import ml_dtypes
import numpy as np

import concourse.bass as bass
import concourse.mybir as mybir
import concourse.tile as tile
from concourse import bacc, bass_isa
from concourse.bass_utils import run_bass_kernel_spmd

F32 = mybir.dt.float32
BF16 = mybir.dt.bfloat16
AX = mybir.AxisListType
OP = mybir.AluOpType
AF = mybir.ActivationFunctionType

N_CORES = 8
B, CIN, COUT, H, W, KS, E = 32, 256, 256, 56, 56, 3, 4
BPC = B // N_CORES  # samples per core
CT = CIN // 128     # cin partition tiles
OTN = COUT // 128   # cout partition tiles
KK = KS * KS
WP = W + 2          # width zero-padded (kj shifts); height handled by clipping
RC = 7              # row chunks per image
RH = H // RC        # rows per chunk
NF = RH * W         # moving-dim elements per matmul
BN_EPS = 1e-5

# conv taps, center first: the center tap covers the full output chunk, so it
# carries start=True and clears every PSUM has_written bit; row-clipped taps
# then accumulate flat sub-slices (= 'same' padding semantics at top/bottom).
TAPS = [(1, 1)] + [(ki, kj) for ki in range(KS) for kj in range(KS)
                   if (ki, kj) != (1, 1)]


def build_bass():
    nc = bacc.Bacc("TRN2", target_bir_lowering=False, debug=False)

    x_d = nc.dram_tensor("x", [BPC, CIN, H, WP], BF16, kind="ExternalInput")
    wt_d = nc.dram_tensor("wt", [E, CIN, OTN, KK, 128], BF16,
                          kind="ExternalInput")
    rwt_d = nc.dram_tensor("rwt", [CIN, E], F32, kind="ExternalInput")
    rb_d = nc.dram_tensor("rb", [E], F32, kind="ExternalInput")
    bnb_d = nc.dram_tensor("bnb", [128, OTN], F32, kind="ExternalInput")
    y_d = nc.dram_tensor("y", [BPC, COUT, H, W], BF16, kind="ExternalOutput")

    x_ap = x_d.ap()
    wt_ap = wt_d.ap()
    rwt_ap = rwt_d.ap()
    rb_ap = rb_d.ap()
    bnb_ap = bnb_d.ap()
    y_ap = y_d.ap()

    with tile.TileContext(nc) as tc:
        with (
            tc.tile_pool(name="wbp", bufs=1) as wbp,
            tc.tile_pool(name="xpp", bufs=1) as xpp,
            tc.tile_pool(name="mwp", bufs=1) as mwp,
            tc.tile_pool(name="otp", bufs=10) as otp,
            tc.tile_pool(name="snp", bufs=1) as snp,
            tc.tile_pool(name="smp", bufs=4) as smp,
            tc.tile_pool(name="psp", bufs=6, space="PSUM") as psp,
        ):
            # ---- persistent tiles ----
            wb = [[wbp.tile([128, OTN, KK, 128], BF16, name=f"wb{e}_{t}",
                            tag=f"wb{e}_{t}")
                   for t in range(CT)] for e in range(E)]
            xp = [[xpp.tile([128, H, WP], BF16, name=f"xp{i}_{t}",
                            tag=f"xp{i}_{t}")
                   for t in range(CT)] for i in range(2)]
            mw = [[mwp.tile([128, OTN, KK, 128], BF16, name=f"mw{i}_{t}",
                            tag=f"mw{i}_{t}")
                   for t in range(CT)] for i in range(2)]
            rwt_sb = [snp.tile([128, E], F32, name=f"rwt{t}", tag=f"rwt{t}")
                      for t in range(CT)]
            rb_bc = snp.tile([128, E], F32, name="rb_bc", tag="rb_bc")
            bnb_sb = snp.tile([128, OTN], F32, name="bnb", tag="bnb")

            # ---- preamble DMAs in priority order: queue order = bandwidth
            # priority. x(0) tiles split across two queues (routing critical
            # path), tiny params next, then the expert bank with the oi=0
            # halves first (mixing consumes them first).
            nc.sync.dma_start(out=xp[0][0][:, 0:28], in_=x_ap[0, 0:128, 0:28, :])
            nc.scalar.dma_start(out=xp[0][0][:, 28:56], in_=x_ap[0, 0:128, 28:56, :])
            nc.sync.dma_start(out=xp[0][1][:, 0:28], in_=x_ap[0, 128:256, 0:28, :])
            nc.gpsimd.dma_start(out=xp[0][1][:, 28:56], in_=x_ap[0, 128:256, 28:56, :])
            for t in range(CT):
                nc.sync.dma_start(out=rwt_sb[t],
                                  in_=rwt_ap[t * 128:(t + 1) * 128, :])
            nc.sync.dma_start(
                out=rb_bc,
                in_=bass.AP(tensor=rb_ap.tensor, offset=0, ap=[[0, 128], [1, E]]))
            nc.sync.dma_start(out=bnb_sb, in_=bnb_ap[:, :])
            for oi in range(OTN):
                for e in range(E):
                    for t in range(CT):
                        nc.sync.dma_start(out=wb[e][t][:, oi],
                                          in_=wt_ap[e, t * 128:(t + 1) * 128, oi])

            # all-zeros per-partition scalar: explicit AP bias for ACT funcs
            # (the float-bias path needs a pre-registered const-AP database)
            zeros1 = snp.tile([128, 1], F32, name="zeros1", tag="zeros1")
            nc.vector.memset(zeros1, 0.0)

            # scratch target for the ACT-side pooled copy (only accum_out used)
            pscr = snp.tile([128, H * W // 2], BF16, name="pscr", tag="pscr")

            # warm-up operands: dependency-gated dummy matmuls keep the PE
            # HAM window busy right before the first real matmul so the real
            # stream starts at full clock (warm_x is touched from `prod` in
            # prep(0) to time the dummies against the routing chain)
            warm_w = snp.tile([128, 128], BF16, name="warm_w", tag="warm_w")
            nc.vector.memset(warm_w, 0.0)
            warm_x = snp.tile([128, NF], BF16, name="warm_x", tag="warm_x")
            nc.vector.memset(warm_x, 0.0)
            warm_ps0 = psp.tile([128, NF], F32, name="warm_ps0", tag="warmps",
                                bufs=1)
            for _ in range(36):
                nc.tensor.matmul(warm_ps0[:], lhsT=warm_w, rhs=warm_x,
                                 start=True, stop=True)

            def prep(s):
                """Routing + weight mixing for sample s (no PE involvement)."""
                i = s % 2
                pooled = [smp.tile([128, 1], F32, name=f"pool{s}_{t}",
                                   tag=f"pool{t}") for t in range(CT)]
                ph = smp.tile([128, 1], F32, name=f"ph{s}", tag="ph")
                # GAP: tile 0 on DVE; tile 1 split into a DVE half and an ACT
                # (Copy + accum_out) half so its reduction finishes ~2x sooner
                # after the tile-1 DMA lands
                nc.vector.reduce_sum(out=pooled[0], in_=xp[i][0][:, :, 1:W + 1],
                                     axis=AX.XY)
                nc.vector.reduce_sum(out=pooled[1],
                                     in_=xp[i][1][:, 0:H // 2, 1:W + 1],
                                     axis=AX.XY)
                nc.scalar.activation(out=pscr, in_=xp[i][1][:, H // 2:H, 1:W + 1],
                                     func=AF.Copy, accum_out=ph)
                prod = smp.tile([128, E], F32, name=f"prod{s}", tag="prod")
                nc.vector.tensor_scalar_mul(prod, rwt_sb[0], pooled[0])
                nc.vector.scalar_tensor_tensor(out=prod, in0=rwt_sb[1],
                                               scalar=pooled[1], in1=prod,
                                               op0=OP.mult, op1=OP.add)
                nc.vector.scalar_tensor_tensor(out=prod, in0=rwt_sb[1],
                                               scalar=ph, in1=prod,
                                               op0=OP.mult, op1=OP.add)
                if s == 0:
                    # touch warm_x from prod, then issue the warm-up matmuls:
                    # they run while the routing tail + mixing completes
                    nc.vector.tensor_copy(warm_x[0:1, 0:E], prod[0:1, :])
                    wps = psp.tile([128, NF], F32, name="warm_ps",
                                   tag="warmps", bufs=1)
                    for _ in range(18):
                        nc.tensor.matmul(wps[:], lhsT=warm_w, rhs=warm_x,
                                         start=True, stop=True)
                lg = smp.tile([128, E], F32, name=f"lg{s}", tag="lg")
                nc.gpsimd.partition_all_reduce(lg, prod, channels=128,
                                               reduce_op=bass_isa.ReduceOp.add)
                nc.vector.scalar_tensor_tensor(out=lg, in0=lg,
                                               scalar=1.0 / (H * W), in1=rb_bc,
                                               op0=OP.mult, op1=OP.add)
                rr = smp.tile([128, E], F32, name=f"rr{s}", tag="rr")
                nc.scalar.activation(out=rr, in_=lg, func=AF.Sigmoid, bias=zeros1)
                # mix per cout half: the first conv of the sample only waits
                # for the oi=0 half of the bank. cin tile 0 accumulates on
                # DVE; tile 1 gets its expert scaling from ACT (scaled Copy)
                # with DVE doing only the adds, so the two chains overlap.
                for oi in range(OTN):
                    nc.vector.tensor_scalar_mul(mw[i][0][:, oi],
                                                wb[0][0][:, oi], rr[:, 0:1])
                    for e in range(1, E):
                        nc.vector.scalar_tensor_tensor(
                            out=mw[i][0][:, oi], in0=wb[e][0][:, oi],
                            scalar=rr[:, e:e + 1], in1=mw[i][0][:, oi],
                            op0=OP.mult, op1=OP.add)
                    ce = [smp.tile([128, KK, 128], BF16, name=f"ce{s}_{oi}_{e}",
                                   tag=f"ce{e}", bufs=2) for e in range(E)]
                    for e in range(E):
                        nc.scalar.activation(out=ce[e], in_=wb[e][1][:, oi],
                                             func=AF.Copy, scale=rr[:, e:e + 1])
                    nc.vector.tensor_add(mw[i][1][:, oi], ce[0], ce[1])
                    nc.vector.tensor_add(mw[i][1][:, oi], mw[i][1][:, oi], ce[2])
                    nc.vector.tensor_add(mw[i][1][:, oi], mw[i][1][:, oi], ce[3])

            def conv(s, oi):
                """One output channel tile of sample s: matmuls + BN/ReLU +
                residual + store."""
                i = s % 2
                o0 = oi * 128
                n_mm = len(TAPS) * CT
                for rc in range(RC):
                    r0 = rc * RH
                    acc = psp.tile([128, NF], F32, name=f"acc{s}_{oi}_{rc}",
                                   tag="acc")
                    k = 0
                    for t in range(CT):
                        # t-major so a chunk's first 9 matmuls only need the
                        # cin-tile-0 mix chain (shaves the sample-0 start)
                        for ki, kj in TAPS:
                            # valid output rows for this tap (clipped at top/
                            # bottom; kj handled by the zero-padded columns)
                            h_lo = max(r0, 1 - ki)
                            h_hi = min(r0 + RH - 1, H - ki)
                            kki = ki * KS + kj
                            nc.tensor.matmul(
                                acc[:, (h_lo - r0) * W:(h_hi - r0 + 1) * W],
                                lhsT=mw[i][t][:, oi, kki, :],
                                rhs=xp[i][t][:, h_lo + ki - 1:h_hi + ki,
                                             kj:kj + W],
                                start=(k == 0), stop=(k == n_mm - 1))
                            k += 1
                    ob = otp.tile([128, NF], BF16, name=f"ob{s}_{oi}_{rc}",
                                  tag="ob")
                    nc.scalar.activation(out=ob[:], in_=acc[:], func=AF.Relu,
                                         bias=bnb_sb[:, oi:oi + 1])
                    ob3 = ob.rearrange("p (a b) -> p a b", a=RH)
                    nc.vector.tensor_add(ob3, ob3,
                                         xp[i][oi][:, r0:r0 + RH, 1:W + 1])
                    nc.sync.dma_start(out=y_ap[s, o0:o0 + 128, r0:r0 + RH, :],
                                      in_=ob3)

            prep(0)
            for s in range(BPC):
                if s + 1 < BPC:
                    j = (s + 1) % 2
                    for t in range(CT):
                        nc.sync.dma_start(
                            out=xp[j][t],
                            in_=x_ap[s + 1, t * 128:(t + 1) * 128, :, :])
                conv(s, 0)
                if s + 1 < BPC:
                    prep(s + 1)
                conv(s, 1)

    nc.compile()
    return nc


_CACHE = {}
_LOCK = threading.Lock()


def prepare_in_maps(inputs):
    """Host-side layout prep (sharding + transposes + dtype casts only)."""
    x = np.asarray(inputs["x"], dtype=np.float32)
    route_w = np.asarray(inputs["route_w"], dtype=np.float32)
    route_b = np.ascontiguousarray(np.asarray(inputs["route_b"], dtype=np.float32))
    expert_w = np.asarray(inputs["expert_w"], dtype=np.float32)
    bn_gamma = np.asarray(inputs["bn_gamma"], dtype=np.float32)
    bn_beta = np.asarray(inputs["bn_beta"], dtype=np.float32)
    bn_mean = np.asarray(inputs["bn_mean"], dtype=np.float32)
    bn_var = np.asarray(inputs["bn_var"], dtype=np.float32)

    # fold BN scale gamma' = gamma/sqrt(var+eps) into the expert bank (it
    # commutes with the linear routing mix); beta' = beta - mean*gamma' is
    # the only BN term left for the device (ACT Relu bias).
    inv = bn_gamma / np.sqrt(bn_var + BN_EPS)
    bank = expert_w * inv[None, :, None, None, None]
    # [E, COUT, CIN, K, K] -> [E, CIN, K, K, COUT] -> [E, CIN, OTN, KK, 128]
    wt = (bank.transpose(0, 2, 3, 4, 1)
          .reshape(E, CIN, KK, OTN, 128)
          .transpose(0, 1, 3, 2, 4))
    wt = np.ascontiguousarray(wt).astype(ml_dtypes.bfloat16)
    rwt = np.ascontiguousarray(route_w.T)  # [CIN, E]
    bnb = np.ascontiguousarray(
        (bn_beta - bn_mean * inv).reshape(OTN, 128).T)  # [128, OTN]

    # width-pad on host: border columns arrive pre-zeroed, so the device DMA
    # is one fully contiguous transfer per (sample, cin-tile)
    xpad = np.zeros((B, CIN, H, WP), dtype=ml_dtypes.bfloat16)
    xpad[:, :, :, 1:W + 1] = x.astype(ml_dtypes.bfloat16)

    return [
        {"x": np.ascontiguousarray(xpad[c * BPC:(c + 1) * BPC]),
         "wt": wt, "rwt": rwt, "rb": route_b, "bnb": bnb}
        for c in range(N_CORES)
    ]


def _get_nc():
    with _LOCK:
        if "nc" not in _CACHE:
            _CACHE["nc"] = build_bass()
        return _CACHE["nc"]


def kernel(**inputs):
    in_maps = prepare_in_maps(inputs)
    nc = _get_nc()
    res = run_bass_kernel_spmd(nc, in_maps, core_ids=list(range(N_CORES)))
    y = np.concatenate([np.asarray(r["y"]) for r in res.results], axis=0)
    return y.astype(np.float32)

